# revision 1
# baseline (speedup 1.0000x reference)
"""Trainium2 Bass kernel for nn_FE_block_3d: four 3D-conv branches (32->8 ch,
3x3x3, SAME) over different triples of the (U,V,H,W) dims of
x [8, 32, 5, 5, 64, 64], each followed by bias + PReLU, concatenated on the
channel axis.

Strategy: pure data parallel over the batch dim (8 cores, B=1 each). Per core,
each conv branch is expressed as banded-Toeplitz matmuls on the tensor engine:

- Phase A (uvx: conv over U,V,H / uvy: conv over U,V,W): contraction packs
  (c_sub=5, u, v) into <=125 partitions, output packs (o_sub=4, u', v') into
  100 PSUM partitions. The (ku, kv) kernel taps live inside the banded
  stationary matrix; the remaining tap (kH or kW) is accumulated in PSUM with
  a shifted rhs access pattern over a zero-padded SBUF copy of x.
- Phases B (uxy: conv over U,H,W) / C (vxy: conv over V,H,W): H is blocked 3x
  into partitions with a 1-halo per block, contraction packs
  (hb, c_sub=8, u-or-v) = 120, output packs (hb', o, u'-or-v') = 120. The
  kU/kV tap is banded in the stationary matrix; kH, kW are PSUM-accumulated
  with shifted rhs.

All matmuls run in float32r (full-rate fp32 path at N>=256). Bias + PReLU are
fused into two post-passes (ScalarE activation + VectorE scalar_tensor_tensor)
reading PSUM, writing SBUF staging that DMAs straight to the output layout.
The padded/blocked x layouts are pre-staged host-side (cheap numpy copies) so
every DMA is a simple <=3-dim access pattern.
"""

import sys

if "/opt/trn_rl_repo" not in sys.path:
    sys.path.insert(0, "/opt/trn_rl_repo")

import numpy as np

C_IN = 32
U = V = 5
H = W = 64
KS = 3
N_CORES = 8

_NC_CACHE = {}


# ---------------------------------------------------------------------------
# Host-side data staging
# ---------------------------------------------------------------------------

def _tap_matrix(n):
    """T[k, i, ip] = 1 if k == i - ip + 1 (SAME conv band), shape [3, n, n]."""
    t = np.zeros((KS, n, n), np.float32)
    for i in range(n):
        for ip in range(n):
            k = i - ip + 1
            if 0 <= k < KS:
                t[k, i, ip] = 1.0
    return t


# phase A v'-split sub-phases: vc0 covers v' in {0,1,2} (v 0..3, c-chunks of
# 6), vc1 covers v' in {3,4} (v 2..4, c-chunks of 8)
VC_A = (
    dict(vps=(0, 1, 2), vs=(0, 1, 2, 3), csz=6, ncc=6),
    dict(vps=(3, 4), vs=(2, 3, 4), csz=8, ncc=4),
)


def _build_wA(w_uvx, w_uvy):
    """[120, 60*120] stationary mats for phase A.
    n = br*30 + (tap*6 + cc for vc0 | 18 + tap*4 + cc for vc1);
    k = c_l*5*nv + u*nv + vi, m = o*5*nvp + u'*nvp + vpi."""
    T5 = _tap_matrix(5)
    out = np.zeros((60, 120, 120), np.float32)
    for br, w in enumerate((w_uvx, w_uvy)):
        for vc, cfg in enumerate(VC_A):
            vps, vs, csz, ncc = cfg["vps"], cfg["vs"], cfg["csz"], cfg["ncc"]
            nv, nvp = len(vs), len(vps)
            Tv = np.zeros((KS, nv, nvp), np.float32)
            for vi, v in enumerate(vs):
                for vpi, vp in enumerate(vps):
                    kv = v - vp + 1
                    if 0 <= kv < KS:
                        Tv[kv, vi, vpi] = 1.0
            for tap in range(KS):
                full = np.einsum("ocab,auy,bvz->cuvoyz",
                                 w[:, :, :, :, tap], T5, Tv)
                for cc in range(ncc):
                    c0 = csz * cc
                    cn = min(csz, 32 - c0)
                    n = br * 30 + (tap * 6 + cc if vc == 0
                                   else 18 + tap * 4 + cc)
                    blk = full[c0:c0 + cn]
                    out[n, :cn * 5 * nv, :8 * 5 * nvp] = blk.reshape(
                        cn * 5 * nv, 8 * 5 * nvp)
    return np.ascontiguousarray(out.transpose(1, 0, 2).reshape(120, 7200))


def _Te():
    """Toeplitz-window band: T[k, e, j] = 1 if k == e - j, shape [3, 5, 3]."""
    t = np.zeros((KS, 5, 3), np.float32)
    for e in range(5):
        for j in range(3):
            k = e - j
            if 0 <= k < KS:
                t[k, e, j] = 1.0
    return t


def _build_wBC(w):
    """[100, 24*120] stationary mats for phases B/C; n = kh*8 + cc.
    k = c_l*25 + u*5 + e (c_l < 4, e = W-window pos), m = o*15 + s'*3 + jw.
    lhsT[k, m] = w[o, 4cc+c_l, s-s'+1, kh, e-jw] (banded in s and in (e,jw))."""
    T5 = _tap_matrix(5)
    Te = _Te()
    out = np.zeros((24, 100, 120), np.float32)
    for kh in range(KS):
        for cc in range(8):
            wsl = w[:, 4 * cc:4 * cc + 4, :, kh, :]  # [o, c4, ks, kw]
            sub = np.einsum("ocab,auy,bej->cueoyj", wsl, T5, Te)
            out[kh * 8 + cc] = sub.reshape(100, 120)
    return np.ascontiguousarray(out.transpose(1, 0, 2).reshape(100, 2880))


def _build_bias(biases, alphas):
    """[128, 18]: unit i -> col 3i = b per partition, 3i+1 = a*b, 3i+2 = a."""
    out = np.zeros((128, 18), np.float32)
    bA = [np.asarray(b, np.float32) for b in biases]
    units = [
        (np.repeat(bA[0], 15), alphas[0]),   # uvx vc0: m=(o,u',v'3)
        (np.repeat(bA[0], 10), alphas[0]),   # uvx vc1: m=(o,u',v'2)
        (np.repeat(bA[1], 15), alphas[1]),   # uvy vc0
        (np.repeat(bA[1], 10), alphas[1]),   # uvy vc1
        (np.repeat(bA[2], 15), alphas[2]),   # uxy: m=(o,u',jw)
        (np.repeat(bA[3], 15), alphas[3]),   # vxy
    ]
    for i, (col, a) in enumerate(units):
        out[: col.size, 3 * i] = col
        out[: col.size, 3 * i + 1] = float(a) * col
        out[: col.size, 3 * i + 2] = float(a)
    return out


def _prep_xa(x):
    """x [32,5,5,64,64] -> xa1 [6, 120, 66, 66] (c6-chunks, v 0..3) and
    xa2 [4, 120, 66, 66] (c8-chunks, v 2..4); h/w zero-padded by 1."""
    xa1 = np.zeros((6, 120, 66, 66), np.float32)
    xa2 = np.zeros((4, 120, 66, 66), np.float32)
    for cc in range(6):
        c0 = 6 * cc
        cn = min(6, 32 - c0)
        blk = x[c0:c0 + cn][:, :, 0:4]
        xa1[cc, :cn * 20, 1:65, 1:65] = blk.reshape(cn * 20, 64, 64)
    for cc in range(4):
        blk = x[8 * cc:8 * cc + 8][:, :, 2:5]
        xa2[cc, :, 1:65, 1:65] = blk.reshape(120, 64, 64)
    return xa1, xa2


def _prep_xbc(x, phase):
    """x [32,5,5,64,64] -> [5 (fold), 8 (cc), 100, 66, 22].
    Partition k = c_l*25 + s*5 + e with s = u (phase 0) or v (phase 1) and e a
    5-wide overlapping W-window (w = 3*wb + e - 1, zero-padded); free dims are
    (h_buf 66 = h+1 zero-padded, wb 22)."""
    xs = x if phase == 0 else np.ascontiguousarray(x.transpose(0, 2, 1, 3, 4))
    xpad = np.zeros((32, 5, 5, 64, 68), np.float32)
    xpad[..., 1:65] = xs
    E = np.stack([xpad[..., e::3][..., :22] for e in range(5)], axis=-1)
    E2 = E.transpose(2, 0, 1, 5, 3, 4)  # (fold, c, s, e, h, wb)
    E3 = E2.reshape(5, 8, 4, 5, 5, 64, 22).reshape(5, 8, 100, 64, 22)
    out = np.zeros((5, 8, 100, 66, 22), np.float32)
    out[:, :, :, 1:65, :] = E3
    return out


# ---------------------------------------------------------------------------
# Bass kernel construction
# ---------------------------------------------------------------------------

def _build_nc(repeat=1):
    import concourse.bass as bass
    import concourse.mybir as mybir
    from concourse import bacc
    from concourse.tile import TileContext

    FR = mybir.dt.float32r
    F32 = mybir.dt.float32
    ALU = mybir.AluOpType

    nc = bacc.Bacc("TRN2", target_bir_lowering=False)
    xa1_d = nc.dram_tensor("xa1", [6, 120, 66 * 66], FR, kind="ExternalInput")
    xa2_d = nc.dram_tensor("xa2", [4, 120, 66 * 66], FR, kind="ExternalInput")
    xb_d = nc.dram_tensor("xb", [5, 8, 100, 66 * 22], FR, kind="ExternalInput")
    xc_d = nc.dram_tensor("xc", [5, 8, 100, 66 * 22], FR, kind="ExternalInput")
    wA_d = nc.dram_tensor("wA", [120, 7200], FR, kind="ExternalInput")
    wB_d = nc.dram_tensor("wB", [100, 2880], FR, kind="ExternalInput")
    wC_d = nc.dram_tensor("wC", [100, 2880], FR, kind="ExternalInput")
    b_d = nc.dram_tensor("bias", [128, 18], F32, kind="ExternalInput")
    # uvx/uvy (channels 0..15) in final layout; uxy/vxy in kernel-native
    # (o, s', jw, fold, h', wb) layouts that the host reassembles.
    out_d = nc.dram_tensor("out", [16, U, V, H, W], F32, kind="ExternalOutput")
    outB_d = nc.dram_tensor("outB", [120, 5 * 64 * 22], F32, kind="ExternalOutput")
    outC_d = nc.dram_tensor("outC", [120, 5 * 64 * 22], F32, kind="ExternalOutput")
    # tiny debug output keeping the warm/touch matmuls live
    dbg_d = nc.dram_tensor("dbg", [128, 4], F32, kind="ExternalOutput")

    PSUM = bass.MemorySpace.PSUM

    with TileContext(nc) as tc:
        with (
            tc.tile_pool(name="bias", bufs=1) as bias_pool,
            tc.tile_pool(name="warm", bufs=1, space=PSUM) as warm_pool,
        ):
            bias_t = bias_pool.tile([128, 18], F32)
            nc.sync.dma_start(bias_t[:], b_d[:])
            # Persistent PSUM bank written only by PE "touch" matmuls. A PE
            # Matmult can carry at most ONE sync wait in walrus codegen, so
            # each freshly-DMA'd tile gets one touch matmul (1 wait each)
            # before the real accumulation groups consume it wait-free.
            warm_t = warm_pool.tile([128, 512], F32)
            for rep in range(repeat):

                # ---------------- Phase A: uvx + uvy ----------------
                # Two v'-sub-phases; K = (c_sub, u, v-range) <= 120,
                # M = (o8, u'5, v'-range) = 120/80; kH-or-kW taps accumulate.
                with (
                    tc.tile_pool(name=f"wA{rep}", bufs=1) as wA_pool,
                    tc.tile_pool(name=f"psA{rep}", bufs=6, space=PSUM) as psA_pool,
                    tc.tile_pool(name=f"stgA{rep}", bufs=2) as stgA_pool,
                    tc.tile_pool(name=f"tqA{rep}", bufs=3) as tqA_pool,
                ):
                    wA_t = wA_pool.tile([120, 60, 120], FR)
                    for un in range(4):
                        nc.sync.dma_start(
                            wA_t[:, 15 * un:15 * (un + 1), :],
                            wA_d[:, 1800 * un:1800 * (un + 1)].rearrange(
                                "k (n m) -> k n m", m=120),
                        )
                    nc.tensor.matmul(warm_t[0:120, 0:120], wA_t[:, 0, :],
                                     wA_t[:, 0, :], start=True, stop=True)
                    for vc in range(2):
                        cfg = VC_A[vc]
                        ncc = cfg["ncc"]
                        nvp = len(cfg["vps"])
                        M = 40 * nvp
                        xd = xa1_d if vc == 0 else xa2_d
                        kps = [120] * 5 + [40] if vc == 0 else [120] * 4
                        with tc.tile_pool(name=f"xa{rep}_{vc}", bufs=1) as xa_pool:
                            xch = []
                            for cc in range(ncc):
                                kp = kps[cc]
                                t = xa_pool.tile([kp, 66, 66], FR,
                                                 tag=f"xa{cc}",
                                                 name=f"xa{rep}_{vc}{cc}")
                                nc.sync.dma_start(
                                    t[:],
                                    xd[cc, 0:kp].rearrange("p (h w) -> p h w",
                                                           w=66),
                                )
                                if cc == 0:
                                    nc.tensor.matmul(warm_t[0:M, 0:256],
                                                     wA_t[0:kp, 0, 0:M],
                                                     t[:, 0:4, 0:64],
                                                     start=True, stop=True)
                                xch.append((t, kp))
                            # wavefront: batches of 6 PSUM groups, chunk-outer
                            groups = [(br, g) for br in range(2)
                                      for g in range(8)]
                            stgs = {}
                            for b0 in range(0, len(groups), 6):
                                batch = groups[b0:b0 + 6]
                                pss = []
                                for br, g in batch:
                                    shape = ([M, 64, 8] if br == 0
                                             else [M, 8, 64])
                                    pss.append(psA_pool.tile(
                                        shape, F32, tag="psA",
                                        name=f"psA{rep}_{vc}{br}{g}"))
                                for cc in range(ncc):
                                    t, kp = xch[cc]
                                    for tap in range(KS):
                                        for i, (br, g) in enumerate(batch):
                                            n = br * 30 + (
                                                tap * 6 + cc if vc == 0
                                                else 18 + tap * 4 + cc)
                                            lhsT = wA_t[0:kp, n, 0:M]
                                            if br == 0:  # uvx: shift along h
                                                rhs = t[0:kp, tap:tap + 64,
                                                        1 + 8 * g:9 + 8 * g]
                                            else:  # uvy: shift along w
                                                rhs = t[0:kp,
                                                        1 + 8 * g:9 + 8 * g,
                                                        tap:tap + 64]
                                            nc.tensor.matmul(
                                                pss[i][:], lhsT, rhs,
                                                start=(cc == 0 and tap == 0),
                                                stop=(cc == ncc - 1
                                                      and tap == 2),
                                            )
                                for i, (br, g) in enumerate(batch):
                                    unit = br * 2 + vc
                                    shape = ([M, 64, 8] if br == 0
                                             else [M, 8, 64])
                                    if br not in stgs:
                                        stgs[br] = stgA_pool.tile(
                                            [M, 64, 64], F32, tag="stgA",
                                            name=f"stgA{rep}_{vc}{br}")
                                    stg = stgs[br]
                                    ps = pss[i]
                                    tq = tqA_pool.tile(
                                        shape, F32, tag="tqA",
                                        name=f"tqA{rep}_{vc}{br}{g}")
                                    ba = bias_t[0:M, 3 * unit + 2:3 * unit + 3]
                                    bq = bias_t[0:M, 3 * unit + 1:3 * unit + 2]
                                    bb = bias_t[0:M, 3 * unit:3 * unit + 1]
                                    # tq = a*z (DVE-only PSUM reads)
                                    nc.vector.tensor_scalar(
                                        tq[:], ps[:], ba, bq,
                                        op0=ALU.mult, op1=ALU.add)
                                    dst = (stg[:, :, 8 * g:8 * g + 8]
                                           if br == 0
                                           else stg[:, 8 * g:8 * g + 8, :])
                                    # out = max(psum + b, tq) = PReLU(z)
                                    nc.vector.scalar_tensor_tensor(
                                        dst, ps[:], bb, tq[:],
                                        op0=ALU.add, op1=ALU.max)
                                    if g == 7:
                                        vp0 = cfg["vps"][0]
                                        dstd = out_d[br * 8:br * 8 + 8, :,
                                                     vp0:vp0 + nvp, :, :]
                                        nc.sync.dma_start(
                                            dstd.rearrange(
                                                "o u v h w -> (o u) v (h w)"),
                                            stg[:],
                                        )

                # ---------------- Phases B (uxy) and C (vxy) ----------------
                # K = (c_l4, u-or-v, e_w5) = 100, M = (o, s', jw3) = 120,
                # 24 PSUM accums (kh x 8 c-chunks) per 352-column group.
                with (
                    tc.tile_pool(name=f"wb{rep}", bufs=2) as w_pool,
                    tc.tile_pool(name=f"xv{rep}", bufs=3) as xv_pool,
                    tc.tile_pool(name=f"psb{rep}", bufs=6,
                                 space=PSUM) as ps_pool,
                    tc.tile_pool(name=f"stgb{rep}", bufs=1) as stg_pool,
                    tc.tile_pool(name=f"tqb{rep}", bufs=3) as tq_pool,
                ):
                    for phase in range(2):
                        xd = xb_d if phase == 0 else xc_d
                        wd = wB_d if phase == 0 else wC_d
                        od = outB_d if phase == 0 else outC_d
                        unit = 4 + phase
                        w_t = w_pool.tile([100, 24, 120], FR, tag="wbc",
                                          name=f"wmat{rep}_{phase}")
                        nc.sync.dma_start(
                            w_t[:], wd[:].rearrange("k (n m) -> k n m", m=120)
                        )
                        nc.tensor.matmul(warm_t[0:120, 0:120], w_t[:, 0, :],
                                         w_t[:, 0, 0:120], start=True,
                                         stop=True)
                        ba = bias_t[0:120, 3 * unit + 2:3 * unit + 3]
                        bq = bias_t[0:120, 3 * unit + 1:3 * unit + 2]
                        bb = bias_t[0:120, 3 * unit:3 * unit + 1]
                        stg = stg_pool.tile([120, 5, 64, 22], F32, tag="stgB",
                                            name=f"stgB{rep}_{phase}")
                        for s in range(5):  # fold dim: v for uxy, u for vxy
                            xv = []
                            for cc in range(8):
                                t = xv_pool.tile([100, 66, 22], FR,
                                                 tag=f"xv{cc}",
                                                 name=f"xv{rep}_{phase}{s}{cc}")
                                nc.sync.dma_start(
                                    t[:],
                                    xd[s, cc].rearrange("p (h w) -> p h w",
                                                        w=22),
                                )
                                if cc == 0:
                                    nc.tensor.matmul(warm_t[0:120, 0:44],
                                                     w_t[:, 0, :],
                                                     t[:, 0:2, :],
                                                     start=True, stop=True)
                                xv.append(t)
                            for tg in range(4):
                                ps = ps_pool.tile([120, 16, 22], F32,
                                                  tag="psB",
                                                  name=f"psB{rep}_{phase}{s}{tg}")
                                for cc in range(8):
                                    for kh in range(KS):
                                        n = kh * 8 + cc
                                        rhs = xv[cc][:,
                                                     16 * tg + kh:
                                                     16 * tg + kh + 16, :]
                                        nc.tensor.matmul(
                                            ps[:], w_t[:, n, :], rhs,
                                            start=(cc == 0 and kh == 0),
                                            stop=(cc == 7 and kh == 2),
                                        )
                                tq = tq_pool.tile([120, 16, 22], F32,
                                                  tag="tqB",
                                                  name=f"tqB{rep}_{phase}{s}{tg}")
                                nc.vector.tensor_scalar(tq[:], ps[:], ba, bq,
                                                        op0=ALU.mult,
                                                        op1=ALU.add)
                                nc.vector.scalar_tensor_tensor(
                                    stg[:, s, 16 * tg:16 * tg + 16, :],
                                    ps[:], bb, tq[:],
                                    op0=ALU.add, op1=ALU.max,
                                )
                            nc.sync.dma_start(
                                od[:, 1408 * s:1408 * (s + 1)],
                                stg[:, s, :, :],
                            )

            # keep the warm/touch matmuls live: read a sliver out to dbg
            with tc.tile_pool(name="dbg", bufs=1) as dbg_pool:
                dbg_t = dbg_pool.tile([128, 4], F32)
                nc.vector.tensor_copy(dbg_t[:], warm_t[:, 0:4])
                nc.sync.dma_start(dbg_d[:], dbg_t[:])

    nc.compile()
    return nc


def _get_nc(repeat=1):
    global _NC_CACHE
    if _NC_CACHE is None:
        _NC_CACHE = {}
    if repeat not in _NC_CACHE:
        _NC_CACHE[repeat] = _build_nc(repeat)
    return _NC_CACHE[repeat]


# ---------------------------------------------------------------------------
# Entry point
# ---------------------------------------------------------------------------

LAST_RESULT = None


def kernel(x, w_uvx, b_uvx, a_uvx, w_uvy, b_uvy, a_uvy,
           w_uxy, b_uxy, a_uxy, w_vxy, b_vxy, a_vxy, _trace=False):
    from concourse.bass_utils import run_bass_kernel_spmd

    global LAST_RESULT
    x = np.ascontiguousarray(np.asarray(x, np.float32))
    wA = _build_wA(np.asarray(w_uvx, np.float32), np.asarray(w_uvy, np.float32))
    wB = _build_wBC(np.asarray(w_uxy, np.float32))
    wC = _build_wBC(np.asarray(w_vxy, np.float32))
    bias = _build_bias(
        (b_uvx, b_uvy, b_uxy, b_vxy),
        [float(np.asarray(a).reshape(-1)[0]) for a in (a_uvx, a_uvy, a_uxy, a_vxy)],
    )

    nc = _get_nc()
    in_maps = []
    for b in range(N_CORES):
        xb_full = x[b]
        xa1, xa2 = _prep_xa(xb_full)
        in_maps.append({
            "xa1": xa1.reshape(6, 120, 66 * 66),
            "xa2": xa2.reshape(4, 120, 66 * 66),
            "xb": _prep_xbc(xb_full, 0).reshape(5, 8, 100, 66 * 22),
            "xc": _prep_xbc(xb_full, 1).reshape(5, 8, 100, 66 * 22),
            "wA": wA, "wB": wB, "wC": wC, "bias": bias,
        })
    res = run_bass_kernel_spmd(nc, in_maps, core_ids=list(range(N_CORES)),
                               trace=_trace)
    LAST_RESULT = res

    full = np.empty((N_CORES, 32, U, V, H, W), np.float32)
    for b, r in enumerate(res.results):
        full[b, 0:16] = r["out"]
        # raw B/C: (o, s', jw, fold, h', wb) -> w = 3*wb + jw
        for ch0, key in ((16, "outB"), (24, "outC")):
            raw = r[key].reshape(8, 5, 3, 5, 64, 22)
            t = np.moveaxis(raw, 2, 5)  # (o, s', fold, h', wb, jw)
            asm = np.ascontiguousarray(t).reshape(8, 5, 5, 64, 66)[..., :64]
            if ch0 == 24:  # vxy: (o, v', u, h, w) -> (o, u, v', h, w)
                asm = asm.transpose(0, 2, 1, 3, 4)
            full[b, ch0:ch0 + 8] = asm
    return full

